# revision 2
# baseline (speedup 1.0000x reference)
"""DepthToSpace (block_size=2, CRD layout) Trainium2 Bass kernel.

x: [16, 256, 128, 128] f32  ->  out: [16, 64, 256, 256] f32
out[b, dd, 2h+i, 2w+k] = x[b, (2i+k)*64 + dd, h, w]

Sharding: batch dim split across 8 NeuronCores (2 examples per core),
no communication. Per core the kernel is a pure reshuffle:

  - partition axis p = (b_local, dd) = 2*64 = 128 partitions.
  - per 16-row input tile: eight 512 KiB HBM->SBUF read DMAs with 2-dim
    APs [[16384,64],[1,2048]] (8 KiB descriptors measured to be the read
    optimum: 4/16/32/64 KiB descriptors, contiguous layouts, 128-partition
    or 3-dim-AP reads all measured slower). b=0 reads on the sync(SP)
    HWDGE ring (even SBUF ports), b=1 on the gpsimd SWDGE ring (odd
    ports) - disjoint port parity per ring.
  - the 2x2 pixel-shuffle interleave runs in SBUF per 8-row output tile:
    the i=0 half on the vector engine (DVE), the i=1 half on the scalar
    (ACT) engine via activation-copy, halving the shuffle's critical
    path (it is otherwise the slowest single engine at ~111 us/rep).
  - per 8-row output tile: four 512 KiB SBUF->HBM write DMAs on the
    scalar ring, one per (b, dd-half) - 32-partition DMAs with 16 KiB
    descriptors on disjoint SDMA-engine quartets. Versus the original
    single 4 MiB write with 32 KiB descriptors this lets the read and
    write streams interleave at the SDMA engines instead of serializing
    (~220 us -> ~150 us per rep measured).
  - in-pool 3 bufs x 32 KiB + out-pool 4 bufs x 16 KiB per partition.
  - _build(reps) emits UNROLL=4 bodies per hardware For_i iteration
    (reps = 4n + r) so tile pools pipeline across repetitions and the
    loop's all-engine semaphore-reset drain is amortized.
"""

import numpy as np

import concourse.bass as bass  # noqa: F401  (registers AP machinery)
import concourse.tile as tile
from concourse import bacc, bass_utils, mybir

# Problem shape (hardcoded per spec).
B, C, H, W = 16, 256, 128, 128
NCORES = 8
BL = B // NCORES  # local batch per core = 2
D = C // 4        # out channels = 64
HT = 16           # input rows per tile
HTO = 8           # input rows per output tile (16 KiB write descriptors)
NT = H // HT      # input tiles per rep = 8
UNROLL = 4

_cached_nc = None


def _build(reps: int = 1, inbufs: int = 3, outbufs: int = 4):
    nc = bacc.Bacc(
        "TRN2",
        target_bir_lowering=False,
        debug=False,
        num_devices=NCORES,
    )
    x = nc.dram_tensor(
        "x", [BL, C, H, W], mybir.dt.float32, kind="ExternalInput"
    ).ap()
    out = nc.dram_tensor(
        "out", [BL, D, 2 * H, 2 * W], mybir.dt.float32, kind="ExternalOutput"
    ).ap()

    # x viewed as [b, dd, cb, h, w] where channel c = cb*64 + dd, cb = 2i+k.
    xr = x.rearrange("b (cb dd) h w -> b dd cb h w", cb=4)

    def body(inp, outp):
        rengs = [nc.sync, nc.gpsimd]
        for t in range(NT):
            h0 = t * HT
            it = inp.tile([128, 4 * HT * W], mybir.dt.float32)
            for b in range(2):
                for cb in range(4):
                    rengs[b].dma_start(
                        it[
                            b * 64 : (b + 1) * 64,
                            cb * HT * W : (cb + 1) * HT * W,
                        ],
                        xr[b, :, cb, h0 : h0 + HT, :],
                    )

            # it[p, (2i+k)*HT*W + h*W + w] -> ot[p, h*4W + i*2W + w*2 + k]
            sv = it[:].rearrange(
                "p (i k h w) -> p i h w k", i=2, k=2, h=HT, w=W
            )
            for t2 in range(HT // HTO):
                ot = outp.tile([128, HTO * 4 * W], mybir.dt.float32)
                dv = ot[:].rearrange(
                    "p (h i w k) -> p i h w k", h=HTO, i=2, w=W, k=2
                )
                nc.vector.tensor_copy(
                    dv[:, 0], sv[:, 0, HTO * t2 : HTO * (t2 + 1)]
                )
                nc.scalar.copy(
                    dv[:, 1], sv[:, 1, HTO * t2 : HTO * (t2 + 1)]
                )
                r0 = 2 * h0 + 2 * HTO * t2
                for b in range(2):
                    for dh in range(2):
                        nc.scalar.dma_start(
                            out[b, dh * 32 : (dh + 1) * 32, r0 : r0 + 2 * HTO, :],
                            ot[b * 64 + dh * 32 : b * 64 + (dh + 1) * 32, :],
                        )

    with tile.TileContext(nc) as tc:
        with tc.tile_pool(name="inp", bufs=inbufs) as inp, tc.tile_pool(
            name="outp", bufs=outbufs
        ) as outp:
            if reps == 1:
                body(inp, outp)
            else:
                n, rem = divmod(reps, UNROLL)
                for _ in range(rem):
                    body(inp, outp)
                if n:
                    with tc.For_i(0, n, 1):
                        for _ in range(UNROLL):
                            body(inp, outp)
    nc.compile()
    return nc


def kernel(x: np.ndarray) -> np.ndarray:
    global _cached_nc
    if _cached_nc is None:
        _cached_nc = _build()
    nc = _cached_nc

    x = np.ascontiguousarray(x, dtype=np.float32)
    in_maps = [
        {"x": np.ascontiguousarray(x[c * BL : (c + 1) * BL])} for c in range(NCORES)
    ]
    res = bass_utils.run_bass_kernel_spmd(nc, in_maps, core_ids=list(range(NCORES)))
    return np.concatenate([r["out"] for r in res.results], axis=0)


# revision 3
# speedup vs baseline: 1.2298x; 1.2298x over previous
"""DepthToSpace (block_size=2, CRD layout) Trainium2 Bass kernel.

x: [16, 256, 128, 128] f32  ->  out: [16, 64, 256, 256] f32
out[b, dd, 2h+i, 2w+k] = x[b, (2i+k)*64 + dd, h, w]

Sharding: batch dim split across 8 NeuronCores (2 examples per core),
no communication. Per core the kernel is a pure reshuffle:
  - partition axis p = (b_local, dd) = 2*64 = 128 partitions
  - per 16-row tile: eight 512 KiB HBM->SBUF read DMAs with clean 2-dim
    access patterns ([[16384,64],[1,2048]], 8 KiB descriptors - measured
    optimum: 4/16/32/64 KiB descriptors, fully-contiguous layouts,
    128-partition reads with 3-dim APs, and 3-ring spreads all measured
    slower), DVE strided copies do the 2x2 pixel-shuffle interleave in
    SBUF, one 4 MiB SBUF->HBM write DMA with 32 KiB contiguous runs per
    partition.
  - DMA traffic is split BY DIRECTION across the queues: all reads are
    round-robined over the sync(SP) and gpsimd rings (disjoint even/odd
    SBUF-port sets per ring), all writes go on the scalar(ACT) ring.
    Both directions then stream concurrently; mixed-direction rings
    measured slower.
  - no explicit cross-phase sync edges: the tile pools (3 input tiles,
    3 output tiles) bound the pipeline depth; Tile's WAR/RAW tracking
    provides the pacing.
  - _build(reps) emits UNROLL=4 kernel bodies per hardware For_i
    iteration (reps = 4n + r: r leading bodies, then For_i(n) x 4).
    Tile pools then pipeline across repetitions, amortizing the loop's
    per-iteration all-engine semaphore-reset drain (For_i resets
    semaphores between barriers every iteration, which otherwise drains
    the DMA pipeline each rep): 218.9 -> 206.7 us/rep measured at
    R=4096. kernel() itself runs reps=1 (unroll inactive).
"""

import numpy as np

import concourse.bass as bass  # noqa: F401  (registers AP machinery)
import concourse.tile as tile
from concourse import bacc, bass_utils, mybir

# Problem shape (hardcoded per spec).
B, C, H, W = 16, 256, 128, 128
NCORES = 8
BL = B // NCORES  # local batch per core = 2
D = C // 4        # out channels = 64
HT = 16           # input rows per tile
NT = H // HT      # tiles per core = 8
UNROLL = 4

_cached_nc = None


def _build(reps: int = 1):
    nc = bacc.Bacc(
        "TRN2",
        target_bir_lowering=False,
        debug=False,
        num_devices=NCORES,
    )
    x = nc.dram_tensor(
        "x", [BL, C, H, W], mybir.dt.float32, kind="ExternalInput"
    ).ap()
    out = nc.dram_tensor(
        "out", [BL, D, 2 * H, 2 * W], mybir.dt.float32, kind="ExternalOutput"
    ).ap()

    # x viewed as [b, dd, cb, h, w] where channel c = cb*64 + dd, cb = 2i+k.
    xr = x.rearrange("b (cb dd) h w -> b dd cb h w", cb=4)

    def body(inp, outp):
        # b=0 reads (partitions 0-63, even SBUF ports) on the sync ring,
        # b=1 reads (partitions 64-127, odd ports) on the gpsimd ring:
        # disjoint port sets per ring, no cross-ring port contention.
        rengs = [nc.sync, nc.gpsimd]
        for t in range(NT):
            h0 = t * HT
            it = inp.tile([128, 4 * HT * W], mybir.dt.float32)
            # one read DMA per (b, cb): clean 2-dim APs [[16384,64],[1,2048]]
            for b in range(2):
                for cb in range(4):
                    rengs[b].dma_start(
                        it[
                            b * 64 : (b + 1) * 64,
                            cb * HT * W : (cb + 1) * HT * W,
                        ],
                        xr[b, :, cb, h0 : h0 + HT, :],
                    )

            ot = outp.tile([128, HT * 4 * W], mybir.dt.float32)
            # ot[p, h*4W + i*2W + w*2 + k] = it[p, (2i+k)*HT*W + h*W + w]
            sv = it[:].rearrange(
                "p (i k h w) -> p i h w k", i=2, k=2, h=HT, w=W
            )
            dv = ot[:].rearrange(
                "p (h i w k) -> p i h w k", h=HT, i=2, w=W, k=2
            )
            for i in range(2):
                nc.vector.tensor_copy(dv[:, i], sv[:, i])

            nc.scalar.dma_start(out[:, :, 2 * h0 : 2 * h0 + 2 * HT, :], ot[:])

    with tile.TileContext(nc) as tc:
        with tc.tile_pool(name="inp", bufs=3) as inp, tc.tile_pool(
            name="outp", bufs=3
        ) as outp:
            if reps == 1:
                body(inp, outp)
            else:
                n, rem = divmod(reps, UNROLL)
                for _ in range(rem):
                    body(inp, outp)
                if n:
                    with tc.For_i(0, n, 1):
                        for _ in range(UNROLL):
                            body(inp, outp)
    nc.compile()
    return nc


def kernel(x: np.ndarray) -> np.ndarray:
    global _cached_nc
    if _cached_nc is None:
        _cached_nc = _build()
    nc = _cached_nc

    x = np.ascontiguousarray(x, dtype=np.float32)
    in_maps = [
        {"x": np.ascontiguousarray(x[c * BL : (c + 1) * BL])} for c in range(NCORES)
    ]
    res = bass_utils.run_bass_kernel_spmd(nc, in_maps, core_ids=list(range(NCORES)))
    return np.concatenate([r["out"] for r in res.results], axis=0)
